# revision 23
# baseline (speedup 1.0000x reference)
"""Self-contained Trainium2 Bass kernel for sliding-window attention.

Problem (hardcoded): B=1, S=8192, dim=1024, H=16 heads, D=64 head dim,
window=512, fp32 I/O.  y = (softmax(mask(rope(xWq^T) rope(xWk^T)^T / 8)) xWv^T) Wo^T

Strategy: sequence-parallel over 8 NeuronCores. Each core owns 1024 query
rows and additionally recomputes K/V for the 512-row halo to its left
(core 0's halo is zero-padded and neutralized via a per-core "vone"
column so no collective is needed).  All matmuls run in bf16 (fp32 PSUM
accumulation); sliding-window causality is applied by extra rank-128
mask matmuls accumulated into the score PSUM before the exp.

Layouts (per core):
  xT    [1024(d), 1536(s)]  x^T shard incl. halo (bf16)
  wq/wk [1024(d), 1024(e')] Wq^T / Wk^T with a per-head even/odd column
                            permutation (rope pair de-interleave)
  Q^T/K^T are produced in [e', s] layout (weight-stationary matmuls) so
  attention needs no transposes: scores are computed transposed,
  S^T[k, q], the softmax denominator comes free from a ones-column
  appended to V, and PV directly yields o^T[e, q] -- the lhsT of the
  output projection.

Perf structure: rope uses sign-baked sin tables (6 DVE ops per etile);
the PE instruction stream is software-pipelined (PV lags scores by 2
slots) with the next etile's projection matmuls woven in as filler;
HBM parameters are laid out chunk-contiguous and DMA'd in consumption
order so the PE starts within a few microseconds.
"""
import sys

sys.path.insert(0, "/opt/trn_rl_repo")

import numpy as np
import ml_dtypes

import concourse.bass as bass
import concourse.mybir as mybir
from concourse import bacc
from concourse.tile import TileContext
from concourse.bass_utils import run_bass_kernel_spmd

BF = ml_dtypes.bfloat16
NCORES = 8
S, DIM, H, D, W = 8192, 1024, 16, 64, 512
SL = S // NCORES          # 1024 own rows / core
SK = SL + W               # 1536 rows incl. left halo
P = 128
NKT = SK // P             # 12 kv tiles
NQB = SL // P             # 8 query tiles
dt = mybir.dt

VA = 80   # V_aug columns: 64 V + 1 ones + pad (32B-aligned stride)

# table column offsets (bf16 columns in the packed tbl parameter)
T_COS, T_SIN = 0, SK
T_UOLD, T_UDIA, T_NEGI = 2 * SK, 2 * SK + P, 2 * SK + 2 * P
T_VONE = 2 * SK + 3 * P
T_PAD = 3472

_compiled = {}


def _build(dbg=False):
    nc = bacc.Bacc("TRN2", target_bir_lowering=False, debug=False,
                   num_devices=NCORES)
    def param(name, shape, dtype=dt.bfloat16, out=False):
        return nc.declare_dram_parameter(name, shape, dtype, isOutput=out)

    xt3 = param("xt3", [3, P, 8, 512])
    wk8 = param("wk8", [8, P, 8, P])
    wq8 = param("wq8", [8, P, 8, P])
    wv2 = param("wv2", [2, P, 8, 512])
    wo2 = param("wo2", [2, P, 8, 512])
    tbl = param("tbl", [P, T_PAD])
    out = param("out", [SL, DIM], dt.float32, out=True)
    dbg_outs = None
    if dbg:
        dbg_outs = {
            "d_qt": param("d_qt", [P, 8 * SL], out=True),
            "d_kt": param("d_kt", [P, 8 * SK], out=True),
            "d_v": param("d_v", [P, NKT * H * VA], out=True),
            "d_ot": param("d_ot", [P, 8 * SL], out=True),
        }

    with TileContext(nc) as tc:
        _body(nc, tc, xt3, wk8, wq8, wv2, wo2, tbl, out, dbg_outs)
    nc.compile()
    return nc


def _brd2(ap_slice, n):
    """Insert a stride-0 middle free dim of size n into a [p, c] AP."""
    return bass.AP(tensor=ap_slice.tensor, offset=ap_slice.offset,
                   ap=[ap_slice.ap[0], [0, n], ap_slice.ap[1]])


def _body(nc, tc, xt3, wk8, wq8, wv2, wo2, tbl, out, dbg_outs=None):
    f32, bf16 = dt.float32, dt.bfloat16

    with tc.tile_pool(name="persist", bufs=1) as per:
        v_sb = per.tile([P, NKT, H, VA], bf16)    # V_aug: [V(64)|ones|pad]
        qt_sb = per.tile([P, 8, SL], bf16)        # Q^T (rope'd, sigma layout)
        kt_sb = per.tile([P, 8, SK], bf16)        # K^T
        ot_sb = per.tile([P, 8, SL], bf16)        # o^T (normalized)
        tbl_sb = per.tile([P, T_PAD], bf16)
        uold_sb = tbl_sb[:, T_UOLD:T_UOLD + P]
        udia_sb = tbl_sb[:, T_UDIA:T_UDIA + P]
        negi_sb = tbl_sb[:, T_NEGI:T_NEGI + P]
        vone_sb = tbl_sb[:, T_VONE:T_VONE + NKT]

        with tc.tile_pool(name="xtp", bufs=1) as xtp, \
             tc.tile_pool(name="wqk", bufs=1) as wqkp, \
             tc.tile_pool(name="proj", bufs=1, space="PSUM") as projp, \
             tc.tile_pool(name="raw", bufs=3) as rawp, \
             tc.tile_pool(name="mro", bufs=1) as mrp, \
             tc.tile_pool(name="pt", bufs=4) as ptp, \
             tc.tile_pool(name="st", bufs=2, space="PSUM") as stp, \
             tc.tile_pool(name="pv", bufs=3, space="PSUM") as pvp, \
             tc.tile_pool(name="osb", bufs=2) as osbp, \
             tc.tile_pool(name="eps", bufs=1) as epsp:
            xt_sb = xtp.tile([P, 8, SK], bf16)
            wk_sb = wqkp.tile([P, 8, DIM], bf16)
            wq_sb = wqkp.tile([P, 8, DIM], bf16)
            wv_sb = wqkp.tile([P, 8, DIM], bf16)
            wo_sb = wqkp.tile([P, 8, DIM], bf16)

            # ---- input DMAs: <=256KB pieces, strict consumption order,
            #      round-robin across the three DGE rings so many HW
            #      queues stream in parallel ----
            xr = [xt3.ap()[sc] for sc in range(3)]
            engs = [nc.sync, nc.scalar, nc.gpsimd]
            pieces = []
            def xt_pieces(sc):
                for dp in range(4):
                    pieces.append((xt_sb[:, 2 * dp:2 * dp + 2,
                                         sc * 512:(sc + 1) * 512],
                                   xr[sc][:, 2 * dp:2 * dp + 2]))
            xt_pieces(0)
            pieces.append((wk_sb[:, :, 0:P], wk8.ap()[0]))
            pieces.append((wq_sb[:, :, 0:P], wq8.ap()[0]))
            pieces.append((tbl_sb[:, 0:1792], tbl[:, 0:1792]))
            pieces.append((tbl_sb[:, 1792:T_PAD], tbl[:, 1792:T_PAD]))
            xt_pieces(1)
            pieces.append((wk_sb[:, :, P:2 * P], wk8.ap()[1]))
            pieces.append((wq_sb[:, :, P:2 * P], wq8.ap()[1]))
            xt_pieces(2)
            for eh in range(2):
                for dp in range(4):
                    pieces.append((wv_sb[:, 2 * dp:2 * dp + 2,
                                         eh * 512:(eh + 1) * 512],
                                   wv2.ap()[eh][:, 2 * dp:2 * dp + 2]))
            for et in range(2, 8):
                pieces.append((wk_sb[:, :, et * P:(et + 1) * P],
                               wk8.ap()[et]))
                pieces.append((wq_sb[:, :, et * P:(et + 1) * P],
                               wq8.ap()[et]))
            for nh in range(2):
                for dp in range(2):
                    pieces.append((wo_sb[:, 4 * dp:4 * dp + 4,
                                         nh * 512:(nh + 1) * 512],
                                   wo2.ap()[nh][:, 4 * dp:4 * dp + 4]))
            for i, (o, n) in enumerate(pieces):
                engs[i % 3].dma_start(out=o, in_=n)

            # ones column of V_aug (per-core halo-validity mask)
            for kt_i in range(NKT):
                nc.vector.tensor_copy(
                    out=v_sb[:, kt_i, :, 64],
                    in_=_brd2(vone_sb[:, kt_i:kt_i + 1], H))

            # ---- projection machinery (weavable units). All tile
            #      allocations happen at EMIT time so pool-slot rotation
            #      matches consumption order. ----
            def proj_units(w_sb, dst, s0, et, pool=None, tag="ps",
                           raw_eng=None):
                """Units for one etile of K or Q projection + rope: per
                chunk [4 matmuls][4 matmuls + psum->sbuf copy], then one
                unit with the 6 DVE rope ops over the full etile."""
                pool = pool or projp
                nsc = SK - s0
                st = {}
                units = []
                for ci, sp in enumerate(range(s0, SK, 512)):
                    def u_a(ci=ci, sp=sp):
                        if "raw" not in st:
                            st["raw"] = rawp.tile([P, SK], bf16,
                                                  name="raw", tag="raw")
                        ps = pool.tile([P, 512], f32, name="ps", tag=tag)
                        st[ci] = ps
                        for d in range(4):
                            nc.tensor.matmul(
                                ps,
                                lhsT=w_sb[:, d, et * P:(et + 1) * P],
                                rhs=xt_sb[:, d, sp:sp + 512],
                                start=(d == 0), stop=False,
                                skip_group_check=True)

                    def u_b(ci=ci, sp=sp):
                        ps = st.pop(ci)
                        for d in range(4, 8):
                            nc.tensor.matmul(
                                ps,
                                lhsT=w_sb[:, d, et * P:(et + 1) * P],
                                rhs=xt_sb[:, d, sp:sp + 512],
                                start=False, stop=(d == 7),
                                skip_group_check=True)
                        c = sp - s0
                        raw = st["raw"]
                        eng = raw_eng or nc.scalar
                        if eng is nc.scalar:
                            eng.copy(out=raw[:, c:c + 512], in_=ps)
                        else:
                            eng.tensor_copy(out=raw[:, c:c + 512], in_=ps)

                    units.append(u_a)
                    units.append(u_b)

                def u_rope():
                    raw = st["raw"]
                    cseg = slice(T_COS + s0, T_COS + SK)
                    sseg = slice(T_SIN + s0, T_SIN + SK)
                    msw = mrp.tile([P, SK], bf16, name="msw", tag="msw")
                    for hb in range(2):
                        E = slice(hb * 64, hb * 64 + 32)
                        O = slice(hb * 64 + 32, hb * 64 + 64)
                        nc.vector.tensor_mul(
                            msw[E, 0:nsc], raw[O, 0:nsc], tbl_sb[O, sseg])
                        nc.vector.tensor_mul(
                            msw[O, 0:nsc], raw[E, 0:nsc], tbl_sb[E, sseg])
                    rw = raw[:, 0:nsc]
                    nc.vector.tensor_mul(rw, rw, tbl_sb[:, cseg])
                    nc.vector.tensor_add(
                        dst[:, et, 0:nsc], rw, msw[:, 0:nsc])

                units.append(u_rope)
                return units

            def run_units(units):
                for u in units:
                    u()

            # ---- upfront: K/Q projections for etiles 0,1 (borrow the
            #      3-deep pv psum slots, idle until attention), ordered
            #      by xt chunk arrival (s0 chunks first, then s512,
            #      s1024), with rope placement keeping the 3-slot raw
            #      pool rotation safe ----
            K0 = proj_units(wk_sb, kt_sb, 0, 0, pool=pvp, tag="pvt")
            K1 = proj_units(wk_sb, kt_sb, 0, 1, pool=pvp, tag="pvt")
            Q0 = proj_units(wq_sb, qt_sb, W, 0, pool=pvp, tag="pvt")
            Q1 = proj_units(wq_sb, qt_sb, W, 1, pool=pvp, tag="pvt")
            run_units([K0[0], K0[1], K1[0], K1[1],
                       K0[2], K0[3], K1[2], K1[3],
                       Q0[0], Q0[1],
                       K0[4], K0[5], K0[6],
                       Q0[2], Q0[3], Q0[4],
                       K1[4], K1[5], K1[6],
                       Q1[0], Q1[1], Q1[2], Q1[3], Q1[4]])

            # ---- V projection (scatter copies on ACT) ----
            for st_i in range(NKT):
                for eh in range(2):
                    ps = pvp.tile([P, 512], f32, name="psv", tag="pvt")
                    for d in range(8):
                        nc.tensor.matmul(
                            ps,
                            lhsT=xt_sb[:, d, st_i * P:(st_i + 1) * P],
                            rhs=wv_sb[:, d, eh * 512:(eh + 1) * 512],
                            start=(d == 0), stop=(d == 7),
                            skip_group_check=True)
                    nc.scalar.copy(
                        out=v_sb[:, st_i, eh * 8:(eh + 1) * 8, 0:64],
                        in_=ps[:, :].rearrange("p (h e) -> p h e", h=8))

            # ---- attention: 2 heads per iter, PV lagged 2 slots, proj
            #      units for etile et+2 woven in as PE filler ----
            pv_t = {}          # head -> [g0 tile, g1 tile]
            p_tiles = {}       # slot idx -> (h, kt, p_t tile, lo, hi)
            slot_no = 0

            def emit_scores(h, kt):
                et, hr = h // 2, (h % 2) * 64
                lo, hi = max(kt - 4, 0), min(kt, 7)
                nqb = hi - lo + 1
                n0 = min(nqb, 4) * P
                kh = kt_sb[hr:hr + 64, et, kt * P:(kt + 1) * P]
                st_ps = stp.tile([P, 640], f32, name="st_ps")
                mms = [(kh, qt_sb[hr:hr + 64, et, lo * P:lo * P + n0],
                        slice(0, n0), True)]
                if nqb == 5:
                    mms.append((kh,
                                qt_sb[hr:hr + 64, et,
                                      (lo + 4) * P:(lo + 5) * P],
                                slice(512, 640), True))
                if kt >= 4:                       # diag mask @ col 0
                    mms.append((udia_sb, negi_sb, slice(0, P), False))
                if kt <= 7:                       # oldest mask @ col kt-lo
                    c = (kt - lo) * P
                    mms.append((uold_sb, negi_sb, slice(c, c + P), False))
                for lhsT, rhs, csl, is_start in mms:
                    nc.tensor.matmul(
                        st_ps[:, csl], lhsT=lhsT, rhs=rhs,
                        start=is_start, stop=not is_start,
                        skip_group_check=True)
                p_t = ptp.tile([P, 640], bf16, name="p_t")
                nc.scalar.activation(
                    out=p_t[:, 0:nqb * P], in_=st_ps[:, 0:nqb * P],
                    func=mybir.ActivationFunctionType.Exp, scale=0.125)
                return p_t, lo, hi

            def emit_pv(h, kt, p_t, lo, hi):
                et, hr = h // 2, (h % 2) * 64
                if h not in pv_t:
                    pv_t[h] = [None, None]
                for g in (0, 1):
                    c0, c1 = max(lo, 4 * g), min(hi, 4 * g + 3)
                    if c0 > c1:
                        continue
                    if pv_t[h][g] is None:
                        pv_t[h][g] = pvp.tile([P, 512], f32, name="pvt",
                                              tag="pvt")
                    nc.tensor.matmul(
                        pv_t[h][g][0:VA, (c0 % 4) * P:(c1 % 4 + 1) * P],
                        lhsT=v_sb[:, kt, h, :],
                        rhs=p_t[:, (c0 - lo) * P:(c1 - lo + 1) * P],
                        start=(kt == 4 * g), stop=(kt == 4 * g + 7),
                        skip_group_check=True)
                for g in (0, 1):
                    if kt == 4 * g + 7:
                        # retire: normalize by the ones-row denominator;
                        # reciprocal runs lane-parallel via a DMA reshape
                        pv = pv_t[h][g]
                        rr = epsp.tile([1, 512], f32, name="rr", tag="rr")
                        bc = epsp.tile([64, 512], f32, name="bc", tag="bc")
                        rcs = epsp.tile([128, 4], f32, name="rcs", tag="rcs")
                        rcr = epsp.tile([128, 4], f32, name="rcr", tag="rcr")
                        nc.vector.tensor_copy(out=rr, in_=pv[64:65, :])
                        nc.sync.dma_start(out=rcs, in_=rr)
                        nc.vector.reciprocal(rcr, rcs)
                        nc.sync.dma_start(out=rr, in_=rcr)
                        nc.gpsimd.partition_broadcast(bc, rr)
                        nc.vector.tensor_mul(
                            ot_sb[hr:hr + 64, et, g * 512:(g + 1) * 512],
                            pv[0:64, :], bc)
                        pv_t[h][g] = None

            LAG = 3
            # all remaining projection work (etiles 2..7), woven into the
            # attention iters front-loaded so each etile lands just in
            # time for the iter that consumes it
            fillers = []
            for fet in range(2, 8):
                fillers += proj_units(wk_sb, kt_sb, 0, fet)
                fillers += proj_units(wq_sb, qt_sb, W, fet,
                                      raw_eng=nc.vector)
            QUOTA = [12, 12, 12, 12, 12, 8, 4, 0]
            done_f = 0
            for et in range(8):
                slots = [(h, kt) for h in (2 * et, 2 * et + 1)
                         for kt in range(NKT)]
                base_f = done_f
                for i, (h, kt) in enumerate(slots):
                    p_t, lo, hi = emit_scores(h, kt)
                    p_tiles[slot_no] = (h, kt, p_t, lo, hi)
                    want = base_f + (i + 1) * QUOTA[et] // len(slots)
                    while done_f < want:
                        fillers[done_f]()
                        done_f += 1
                    if slot_no - LAG in p_tiles:
                        ph, pkt, pp, plo, phi = p_tiles.pop(slot_no - LAG)
                        emit_pv(ph, pkt, pp, plo, phi)
                    slot_no += 1
                if et == 7:   # drain the lagged PV slots
                    for s in sorted(p_tiles):
                        ph, pkt, pp, plo, phi = p_tiles.pop(s)
                        emit_pv(ph, pkt, pp, plo, phi)

            if dbg_outs is not None:
                nc.sync.dma_start(out=dbg_outs["d_qt"][:, :], in_=qt_sb[:, :, :])
                nc.sync.dma_start(out=dbg_outs["d_kt"][:, :], in_=kt_sb[:, :, :])
                nc.sync.dma_start(out=dbg_outs["d_v"][:, :],
                                  in_=v_sb[:, :, :, :])
                nc.sync.dma_start(out=dbg_outs["d_ot"][:, :], in_=ot_sb[:, :, :])

            # ---- output projection ----
            for qt_i in range(NQB):
                for nh in range(2):
                    ps = pvp.tile([P, 512], f32, name="pso", tag="pvt")
                    for p in range(8):
                        nc.tensor.matmul(
                            ps,
                            lhsT=ot_sb[:, p, qt_i * P:(qt_i + 1) * P],
                            rhs=wo_sb[:, p, nh * 512:(nh + 1) * 512],
                            start=(p == 0), stop=(p == 7),
                            skip_group_check=True)
                    o_sb = osbp.tile([P, 512], f32, name="o_sb")
                    nc.vector.tensor_copy(o_sb, ps)
                    nc.sync.dma_start(
                        out=out[qt_i * P:(qt_i + 1) * P,
                                nh * 512:(nh + 1) * 512],
                        in_=o_sb)


def _prep_inputs(x, Wq, Wk, Wv, Wo):
    """Host-side shard/layout prep -> list of 8 per-core input dicts."""
    x2 = np.ascontiguousarray(x.reshape(S, DIM).astype(np.float32))
    sigma = np.zeros(DIM, dtype=np.int64)
    for h in range(H):
        j = np.arange(32)
        sigma[h * 64 + j] = h * 64 + 2 * j
        sigma[h * 64 + 32 + j] = h * 64 + 2 * j + 1
    wq_h = np.ascontiguousarray(Wq.T[:, sigma]).astype(np.float32)
    wk_h = np.ascontiguousarray(Wk.T[:, sigma]).astype(np.float32)
    wv_h = np.ascontiguousarray(Wv.T).astype(np.float32)
    wo_h = np.ascontiguousarray(Wo.T).astype(np.float32)

    def chunk4(w, ncol):
        nc_ = DIM // ncol
        return np.ascontiguousarray(
            w.reshape(8, P, nc_, ncol).transpose(2, 1, 0, 3)).astype(BF)

    wk8_h = chunk4(wk_h, P)
    wq8_h = chunk4(wq_h, P)
    wv2_h = chunk4(wv_h, 512)
    wo2_h = chunk4(wo_h, 512)

    jj = np.arange(P)
    uold_h = (jj[None, :] <= jj[:, None]).astype(np.float32)
    udia_h = (jj[None, :] >= jj[:, None] + 1).astype(np.float32)
    negi_h = -1e6 * np.eye(P, dtype=np.float32)

    inv_freq = 1.0 / (10000.0 ** (np.arange(0, D, 2, dtype=np.float32) / D))
    xT = x2.T  # [DIM, S]
    # sign-baked sin table: +sin on even-half rows (E), -sin on odd-half (O)
    sign = np.where((np.arange(P) % 64) < 32, 1.0, -1.0).astype(np.float32)

    in_maps = []
    for core in range(NCORES):
        lo = core * SL - W
        xsh = np.zeros((DIM, SK), dtype=np.float32)
        if lo < 0:
            xsh[:, W:] = xT[:, :SL]
        else:
            xsh[:, :] = xT[:, lo:lo + SK]
        xt3_h = np.ascontiguousarray(
            xsh.reshape(8, P, 3, 512).transpose(2, 1, 0, 3)).astype(BF)
        pos = np.arange(lo, lo + SK, dtype=np.float32)
        ang = pos[None, :] * inv_freq[:, None]          # [32, SK]
        ropc = np.tile(np.cos(ang), (4, 1))             # [128, SK]
        rops = np.tile(np.sin(ang), (4, 1)) * sign[:, None]
        vone = (pos.reshape(NKT, P).T >= 0).astype(np.float32)
        tbl_h = np.zeros((P, T_PAD), dtype=np.float32)
        tbl_h[:, T_COS:T_COS + SK] = ropc
        tbl_h[:, T_SIN:T_SIN + SK] = rops
        tbl_h[:, T_UOLD:T_UOLD + P] = uold_h
        tbl_h[:, T_UDIA:T_UDIA + P] = udia_h
        tbl_h[:, T_NEGI:T_NEGI + P] = negi_h
        tbl_h[:, T_VONE:T_VONE + NKT] = vone
        in_maps.append({
            "xt3": xt3_h,
            "wk8": wk8_h, "wq8": wq8_h, "wv2": wv2_h, "wo2": wo2_h,
            "tbl": tbl_h.astype(BF),
        })
    return in_maps


def kernel(x, Wq, Wk, Wv, Wo, window_size, _trace=False, _trace_kwargs=None):
    assert int(window_size) == W
    if "nc" not in _compiled:
        _compiled["nc"] = _build()
    nc = _compiled["nc"]
    in_maps = _prep_inputs(np.asarray(x), np.asarray(Wq), np.asarray(Wk),
                           np.asarray(Wv), np.asarray(Wo))
    res = run_bass_kernel_spmd(nc, in_maps, core_ids=list(range(NCORES)),
                               trace=_trace, **(_trace_kwargs or {}))
    outp = np.concatenate([res.results[c]["out"] for c in range(NCORES)],
                          axis=0)
    _compiled["last_result"] = res
    return outp.reshape(1, S, DIM).astype(np.float32)


if __name__ == "__main__":
    np.random.seed(0)
    x = np.random.randn(1, S, DIM).astype(np.float32)
    sd = 1.0 / np.sqrt(DIM)
    ws = [np.random.randn(DIM, DIM).astype(np.float32) * sd for _ in range(4)]
    y = kernel(x, *ws, window_size=W)
    print("kernel output", y.shape, y.dtype, np.abs(y).max())


# revision 28
# speedup vs baseline: 1.2161x; 1.2161x over previous
"""Self-contained Trainium2 Bass kernel for sliding-window attention.

Problem (hardcoded): B=1, S=8192, dim=1024, H=16 heads, D=64 head dim,
window=512, fp32 I/O.  y = (softmax(mask(rope(xWq^T) rope(xWk^T)^T / 8)) xWv^T) Wo^T

Strategy: sequence-parallel over 8 NeuronCores. Each core owns 1024 query
rows and additionally recomputes K/V for the 512-row halo to its left
(core 0's halo is zero-padded and neutralized via a per-core "vone"
column so no collective is needed).  All matmuls run in bf16 (fp32 PSUM
accumulation); sliding-window causality is applied by extra rank-128
mask matmuls accumulated into the score PSUM before the exp.

Layouts (per core):
  xT    [1024(d), 1536(s)]  x^T shard incl. halo (bf16)
  wq/wk [1024(d), 1024(e')] Wq^T / Wk^T with a per-head even/odd column
                            permutation (rope pair de-interleave)
  Q^T/K^T are produced in [e', s] layout (weight-stationary matmuls) so
  attention needs no transposes: scores are computed transposed,
  S^T[k, q], the softmax denominator comes free from a ones-column
  appended to V, and PV directly yields o^T[e, q] -- the lhsT of the
  output projection.

Perf structure: rope uses sign-baked sin tables (6 DVE ops per etile);
the PE instruction stream is software-pipelined (PV lags scores by 2
slots) with the next etile's projection matmuls woven in as filler;
HBM parameters are laid out chunk-contiguous and DMA'd in consumption
order so the PE starts within a few microseconds.
"""
import sys

sys.path.insert(0, "/opt/trn_rl_repo")

import numpy as np
import ml_dtypes

import concourse.bass as bass
import concourse.mybir as mybir
from concourse import bacc
from concourse.tile import TileContext
from concourse.bass_utils import run_bass_kernel_spmd

BF = ml_dtypes.bfloat16
NCORES = 8
S, DIM, H, D, W = 8192, 1024, 16, 64, 512
SL = S // NCORES          # 1024 own rows / core
SK = SL + W               # 1536 rows incl. left halo
P = 128
NKT = SK // P             # 12 kv tiles
NQB = SL // P             # 8 query tiles
dt = mybir.dt

VA = 80   # V_aug columns: 64 V + 1 ones + pad (32B-aligned stride)

# table column offsets (bf16 columns in the packed tbl parameter)
T_COS, T_SIN = 0, SK
T_UOLD, T_UDIA, T_NEGI = 2 * SK, 2 * SK + P, 2 * SK + 2 * P
T_VONE = 2 * SK + 3 * P
T_PAD = 3472

_compiled = {}


def _build(dbg=False):
    nc = bacc.Bacc("TRN2", target_bir_lowering=False, debug=False,
                   num_devices=NCORES)
    def param(name, shape, dtype=dt.bfloat16, out=False):
        return nc.declare_dram_parameter(name, shape, dtype, isOutput=out)

    xt3 = param("xt3", [3, P, 8, 512])
    wk8 = param("wk8", [8, P, 8, P])
    wq8 = param("wq8", [8, P, 8, P])
    wv2 = param("wv2", [2, P, 8, 512])
    wo2 = param("wo2", [2, P, 8, 512])
    tbl = param("tbl", [P, T_PAD])
    out = param("out", [SL, DIM], dt.float32, out=True)
    dbg_outs = None
    if dbg:
        dbg_outs = {
            "d_qt": param("d_qt", [P, 8 * SL], out=True),
            "d_kt": param("d_kt", [P, 8 * SK], out=True),
            "d_v": param("d_v", [P, NKT * H * VA], out=True),
            "d_ot": param("d_ot", [P, 8 * SL], out=True),
        }

    with TileContext(nc) as tc:
        _body(nc, tc, xt3, wk8, wq8, wv2, wo2, tbl, out, dbg_outs)
    nc.compile()
    return nc


def _brd2(ap_slice, n):
    """Insert a stride-0 middle free dim of size n into a [p, c] AP."""
    return bass.AP(tensor=ap_slice.tensor, offset=ap_slice.offset,
                   ap=[ap_slice.ap[0], [0, n], ap_slice.ap[1]])


def _body(nc, tc, xt3, wk8, wq8, wv2, wo2, tbl, out, dbg_outs=None):
    f32, bf16 = dt.float32, dt.bfloat16

    with tc.tile_pool(name="persist", bufs=1) as per:
        v_sb = per.tile([P, NKT, H, VA], bf16)    # V_aug: [V(64)|ones|pad]
        qt_sb = per.tile([P, 8, SL], bf16)        # Q^T (rope'd, sigma layout)
        kt_sb = per.tile([P, 8, SK], bf16)        # K^T
        ot_sb = per.tile([P, 8, SL], bf16)        # o^T (normalized)
        tbl_sb = per.tile([P, T_PAD], bf16)
        uold_sb = tbl_sb[:, T_UOLD:T_UOLD + P]
        udia_sb = tbl_sb[:, T_UDIA:T_UDIA + P]
        negi_sb = tbl_sb[:, T_NEGI:T_NEGI + P]
        vone_sb = tbl_sb[:, T_VONE:T_VONE + NKT]

        with tc.tile_pool(name="xtp", bufs=1) as xtp, \
             tc.tile_pool(name="wqk", bufs=1) as wqkp, \
             tc.tile_pool(name="proj", bufs=1, space="PSUM") as projp, \
             tc.tile_pool(name="raw", bufs=2) as rawp, \
             tc.tile_pool(name="mro", bufs=1) as mrp, \
             tc.tile_pool(name="pt", bufs=3) as ptp, \
             tc.tile_pool(name="st", bufs=2, space="PSUM") as stp, \
             tc.tile_pool(name="pv", bufs=3, space="PSUM") as pvp, \
             tc.tile_pool(name="osb", bufs=2) as osbp, \
             tc.tile_pool(name="eps", bufs=1) as epsp:
            xt_sb = xtp.tile([P, 8, SK], bf16)
            wk_sb = wqkp.tile([P, 8, DIM], bf16)
            wq_sb = wqkp.tile([P, 8, DIM], bf16)
            wv_sb = wqkp.tile([P, 8, DIM], bf16)
            wo_sb = wqkp.tile([P, 8, DIM], bf16)

            # ---- input DMAs: per-ring ordered priority lists (rings
            #      drain independently; sizes balanced per ring) ----
            xr = [xt3.ap()[sc] for sc in range(3)]
            nc.sync.dma_start(out=xt_sb[:, 0:4, 0:512], in_=xr[0][:, 0:4])
            nc.sync.dma_start(out=xt_sb[:, 4:8, 0:512], in_=xr[0][:, 4:8])
            nc.scalar.dma_start(out=wk_sb[:, :, 0:P], in_=wk8.ap()[0])
            nc.gpsimd.dma_start(out=tbl_sb, in_=tbl[:, :])
            nc.sync.dma_start(out=xt_sb[:, 0:4, 512:1024], in_=xr[1][:, 0:4])
            nc.sync.dma_start(out=xt_sb[:, 4:8, 512:1024], in_=xr[1][:, 4:8])
            nc.scalar.dma_start(out=wq_sb[:, :, 0:P], in_=wq8.ap()[0])
            nc.sync.dma_start(out=xt_sb[:, 0:4, 1024:1536], in_=xr[2][:, 0:4])
            nc.sync.dma_start(out=xt_sb[:, 4:8, 1024:1536], in_=xr[2][:, 4:8])
            nc.scalar.dma_start(out=wk_sb[:, :, P:2 * P], in_=wk8.ap()[1])
            nc.scalar.dma_start(out=wq_sb[:, :, P:2 * P], in_=wq8.ap()[1])
            for eh in range(2):
                for dh in range(2):
                    nc.gpsimd.dma_start(
                        out=wv_sb[:, 4 * dh:4 * dh + 4,
                                  eh * 512:(eh + 1) * 512],
                        in_=wv2.ap()[eh][:, 4 * dh:4 * dh + 4])
            for et in range(2, 8):
                eng = nc.sync if et % 2 == 0 else nc.scalar
                eng.dma_start(out=wk_sb[:, :, et * P:(et + 1) * P],
                              in_=wk8.ap()[et])
                eng.dma_start(out=wq_sb[:, :, et * P:(et + 1) * P],
                              in_=wq8.ap()[et])
            for nh in range(2):
                nc.sync.dma_start(out=wo_sb[:, :, nh * 512:(nh + 1) * 512],
                                  in_=wo2.ap()[nh])

            # ones column of V_aug (per-core halo-validity mask)
            for kt_i in range(NKT):
                nc.vector.tensor_copy(
                    out=v_sb[:, kt_i, :, 64],
                    in_=_brd2(vone_sb[:, kt_i:kt_i + 1], H))

            # ---- projection machinery (weavable units). All tile
            #      allocations happen at EMIT time so pool-slot rotation
            #      matches consumption order. ----
            def proj_units(w_sb, dst, s0, et, pool=None, tag="ps",
                           raw_eng=None):
                """Units for one etile of K or Q projection + rope: per
                chunk [4 matmuls][4 matmuls + psum->sbuf copy], then one
                unit with the 6 DVE rope ops over the full etile."""
                pool = pool or projp
                nsc = SK - s0
                st = {}
                units = []
                for ci, sp in enumerate(range(s0, SK, 512)):
                    def u_a(ci=ci, sp=sp):
                        if "raw" not in st:
                            st["raw"] = rawp.tile([P, SK], bf16,
                                                  name="raw", tag="raw")
                        ps = pool.tile([P, 512], f32, name="ps", tag=tag)
                        st[ci] = ps
                        for d in range(4):
                            nc.tensor.matmul(
                                ps,
                                lhsT=w_sb[:, d, et * P:(et + 1) * P],
                                rhs=xt_sb[:, d, sp:sp + 512],
                                start=(d == 0), stop=False,
                                skip_group_check=True)

                    def u_b(ci=ci, sp=sp):
                        ps = st.pop(ci)
                        for d in range(4, 8):
                            nc.tensor.matmul(
                                ps,
                                lhsT=w_sb[:, d, et * P:(et + 1) * P],
                                rhs=xt_sb[:, d, sp:sp + 512],
                                start=False, stop=(d == 7),
                                skip_group_check=True)
                        c = sp - s0
                        raw = st["raw"]
                        eng = raw_eng or nc.scalar
                        if eng is nc.scalar:
                            eng.copy(out=raw[:, c:c + 512], in_=ps)
                        else:
                            eng.tensor_copy(out=raw[:, c:c + 512], in_=ps)

                    units.append(u_a)
                    units.append(u_b)

                def u_rope():
                    raw = st["raw"]
                    cseg = slice(T_COS + s0, T_COS + SK)
                    sseg = slice(T_SIN + s0, T_SIN + SK)
                    msw = mrp.tile([P, SK], bf16, name="msw", tag="msw")
                    for hb in range(2):
                        E = slice(hb * 64, hb * 64 + 32)
                        O = slice(hb * 64 + 32, hb * 64 + 64)
                        nc.vector.tensor_mul(
                            msw[E, 0:nsc], raw[O, 0:nsc], tbl_sb[O, sseg])
                        nc.vector.tensor_mul(
                            msw[O, 0:nsc], raw[E, 0:nsc], tbl_sb[E, sseg])
                    rw = raw[:, 0:nsc]
                    nc.vector.tensor_mul(rw, rw, tbl_sb[:, cseg])
                    nc.vector.tensor_add(
                        dst[:, et, 0:nsc], rw, msw[:, 0:nsc])

                units.append(u_rope)
                return units

            def run_units(units):
                for u in units:
                    u()

            # ---- upfront: K/Q projections for etiles 0,1 (borrow the
            #      3-deep pv psum slots, idle until attention) ----
            run_units(proj_units(wk_sb, kt_sb, 0, 0, pool=pvp, tag="pvt"))
            run_units(proj_units(wq_sb, qt_sb, W, 0, pool=pvp, tag="pvt"))
            run_units(proj_units(wk_sb, kt_sb, 0, 1, pool=pvp, tag="pvt"))
            run_units(proj_units(wq_sb, qt_sb, W, 1, pool=pvp, tag="pvt"))

            # ---- V projection (scatter copies on ACT) ----
            for st_i in range(NKT):
                for eh in range(2):
                    ps = pvp.tile([P, 512], f32, name="psv", tag="pvt")
                    for d in range(8):
                        nc.tensor.matmul(
                            ps,
                            lhsT=xt_sb[:, d, st_i * P:(st_i + 1) * P],
                            rhs=wv_sb[:, d, eh * 512:(eh + 1) * 512],
                            start=(d == 0), stop=(d == 7),
                            skip_group_check=True)
                    nc.scalar.copy(
                        out=v_sb[:, st_i, eh * 8:(eh + 1) * 8, 0:64],
                        in_=ps[:, :].rearrange("p (h e) -> p h e", h=8))

            # ---- attention: 2 heads per iter, PV lagged 2 slots, proj
            #      units for etile et+2 woven in as PE filler ----
            pv_t = {}          # head -> [g0 tile, g1 tile]
            p_tiles = {}       # slot idx -> (h, kt, p_t tile, lo, hi)
            slot_no = 0

            def emit_scores(h, kt):
                et, hr = h // 2, (h % 2) * 64
                lo, hi = max(kt - 4, 0), min(kt, 7)
                nqb = hi - lo + 1
                n0 = min(nqb, 4) * P
                kh = kt_sb[hr:hr + 64, et, kt * P:(kt + 1) * P]
                st_ps = stp.tile([P, 640], f32, name="st_ps")
                mms = [(kh, qt_sb[hr:hr + 64, et, lo * P:lo * P + n0],
                        slice(0, n0), True)]
                if nqb == 5:
                    mms.append((kh,
                                qt_sb[hr:hr + 64, et,
                                      (lo + 4) * P:(lo + 5) * P],
                                slice(512, 640), True))
                if kt >= 4:                       # diag mask @ col 0
                    mms.append((udia_sb, negi_sb, slice(0, P), False))
                if kt <= 7:                       # oldest mask @ col kt-lo
                    c = (kt - lo) * P
                    mms.append((uold_sb, negi_sb, slice(c, c + P), False))
                for lhsT, rhs, csl, is_start in mms:
                    nc.tensor.matmul(
                        st_ps[:, csl], lhsT=lhsT, rhs=rhs,
                        start=is_start, stop=not is_start,
                        skip_group_check=True)
                p_t = ptp.tile([P, 640], bf16, name="p_t")
                nc.scalar.activation(
                    out=p_t[:, 0:nqb * P], in_=st_ps[:, 0:nqb * P],
                    func=mybir.ActivationFunctionType.Exp, scale=0.125)
                return p_t, lo, hi

            def emit_pv(h, kt, p_t, lo, hi):
                et, hr = h // 2, (h % 2) * 64
                if h not in pv_t:
                    pv_t[h] = [None, None]
                for g in (0, 1):
                    c0, c1 = max(lo, 4 * g), min(hi, 4 * g + 3)
                    if c0 > c1:
                        continue
                    if pv_t[h][g] is None:
                        pv_t[h][g] = pvp.tile([P, 512], f32, name="pvt",
                                              tag="pvt")
                    nc.tensor.matmul(
                        pv_t[h][g][0:VA, (c0 % 4) * P:(c1 % 4 + 1) * P],
                        lhsT=v_sb[:, kt, h, :],
                        rhs=p_t[:, (c0 - lo) * P:(c1 - lo + 1) * P],
                        start=(kt == 4 * g), stop=(kt == 4 * g + 7),
                        skip_group_check=True)
                for g in (0, 1):
                    if kt == 4 * g + 7:
                        # retire: normalize by the ones-row denominator;
                        # reciprocal runs lane-parallel via a DMA reshape
                        pv = pv_t[h][g]
                        rc = epsp.tile([1, 512], f32, name="rc", tag="rc")
                        bc = epsp.tile([64, 512], f32, name="bc", tag="bc")
                        rcs = epsp.tile([128, 4], f32, name="rcs", tag="rcs")
                        rcr = epsp.tile([128, 4], f32, name="rcr", tag="rcr")
                        rrow = epsp.tile([1, 512], f32, name="rrow",
                                         tag="rrow")
                        nc.vector.tensor_copy(out=rrow, in_=pv[64:65, :])
                        nc.sync.dma_start(out=rcs, in_=rrow)
                        nc.vector.reciprocal(rcr, rcs)
                        nc.sync.dma_start(out=rc, in_=rcr)
                        nc.gpsimd.partition_broadcast(bc, rc)
                        nc.vector.tensor_mul(
                            ot_sb[hr:hr + 64, et, g * 512:(g + 1) * 512],
                            pv[0:64, :], bc)
                        pv_t[h][g] = None

            LAG = 2
            for et in range(8):
                fillers = []
                if et + 2 < 8:
                    fillers += proj_units(wk_sb, kt_sb, 0, et + 2)
                    fillers += proj_units(wq_sb, qt_sb, W, et + 2,
                                          raw_eng=nc.vector)
                slots = [(h, kt) for h in (2 * et, 2 * et + 1)
                         for kt in range(NKT)]
                nf = len(fillers)
                done_f = 0
                for i, (h, kt) in enumerate(slots):
                    p_t, lo, hi = emit_scores(h, kt)
                    p_tiles[slot_no] = (h, kt, p_t, lo, hi)
                    want = (i + 1) * nf // len(slots)
                    while done_f < want:
                        fillers[done_f]()
                        done_f += 1
                    if slot_no - LAG in p_tiles:
                        ph, pkt, pp, plo, phi = p_tiles.pop(slot_no - LAG)
                        emit_pv(ph, pkt, pp, plo, phi)
                    slot_no += 1
                if et == 7:   # drain the lagged PV slots
                    for s in sorted(p_tiles):
                        ph, pkt, pp, plo, phi = p_tiles.pop(s)
                        emit_pv(ph, pkt, pp, plo, phi)

            if dbg_outs is not None:
                nc.sync.dma_start(out=dbg_outs["d_qt"][:, :], in_=qt_sb[:, :, :])
                nc.sync.dma_start(out=dbg_outs["d_kt"][:, :], in_=kt_sb[:, :, :])
                nc.sync.dma_start(out=dbg_outs["d_v"][:, :],
                                  in_=v_sb[:, :, :, :])
                nc.sync.dma_start(out=dbg_outs["d_ot"][:, :], in_=ot_sb[:, :, :])

            # ---- output projection ----
            for qt_i in range(NQB):
                for nh in range(2):
                    ps = pvp.tile([P, 512], f32, name="pso", tag="pvt")
                    for p in range(8):
                        nc.tensor.matmul(
                            ps,
                            lhsT=ot_sb[:, p, qt_i * P:(qt_i + 1) * P],
                            rhs=wo_sb[:, p, nh * 512:(nh + 1) * 512],
                            start=(p == 0), stop=(p == 7),
                            skip_group_check=True)
                    o_sb = osbp.tile([P, 512], f32, name="o_sb")
                    nc.vector.tensor_copy(o_sb, ps)
                    nc.sync.dma_start(
                        out=out[qt_i * P:(qt_i + 1) * P,
                                nh * 512:(nh + 1) * 512],
                        in_=o_sb)


def _prep_inputs(x, Wq, Wk, Wv, Wo):
    """Host-side shard/layout prep -> list of 8 per-core input dicts."""
    x2 = np.ascontiguousarray(x.reshape(S, DIM).astype(np.float32))
    sigma = np.zeros(DIM, dtype=np.int64)
    for h in range(H):
        j = np.arange(32)
        sigma[h * 64 + j] = h * 64 + 2 * j
        sigma[h * 64 + 32 + j] = h * 64 + 2 * j + 1
    wq_h = np.ascontiguousarray(Wq.T[:, sigma]).astype(np.float32)
    wk_h = np.ascontiguousarray(Wk.T[:, sigma]).astype(np.float32)
    wv_h = np.ascontiguousarray(Wv.T).astype(np.float32)
    wo_h = np.ascontiguousarray(Wo.T).astype(np.float32)

    def chunk4(w, ncol):
        nc_ = DIM // ncol
        return np.ascontiguousarray(
            w.reshape(8, P, nc_, ncol).transpose(2, 1, 0, 3)).astype(BF)

    wk8_h = chunk4(wk_h, P)
    wq8_h = chunk4(wq_h, P)
    wv2_h = chunk4(wv_h, 512)
    wo2_h = chunk4(wo_h, 512)

    jj = np.arange(P)
    uold_h = (jj[None, :] <= jj[:, None]).astype(np.float32)
    udia_h = (jj[None, :] >= jj[:, None] + 1).astype(np.float32)
    negi_h = -1e6 * np.eye(P, dtype=np.float32)

    inv_freq = 1.0 / (10000.0 ** (np.arange(0, D, 2, dtype=np.float32) / D))
    xT = x2.T  # [DIM, S]
    # sign-baked sin table: +sin on even-half rows (E), -sin on odd-half (O)
    sign = np.where((np.arange(P) % 64) < 32, 1.0, -1.0).astype(np.float32)

    in_maps = []
    for core in range(NCORES):
        lo = core * SL - W
        xsh = np.zeros((DIM, SK), dtype=np.float32)
        if lo < 0:
            xsh[:, W:] = xT[:, :SL]
        else:
            xsh[:, :] = xT[:, lo:lo + SK]
        xt3_h = np.ascontiguousarray(
            xsh.reshape(8, P, 3, 512).transpose(2, 1, 0, 3)).astype(BF)
        pos = np.arange(lo, lo + SK, dtype=np.float32)
        ang = pos[None, :] * inv_freq[:, None]          # [32, SK]
        ropc = np.tile(np.cos(ang), (4, 1))             # [128, SK]
        rops = np.tile(np.sin(ang), (4, 1)) * sign[:, None]
        vone = (pos.reshape(NKT, P).T >= 0).astype(np.float32)
        tbl_h = np.zeros((P, T_PAD), dtype=np.float32)
        tbl_h[:, T_COS:T_COS + SK] = ropc
        tbl_h[:, T_SIN:T_SIN + SK] = rops
        tbl_h[:, T_UOLD:T_UOLD + P] = uold_h
        tbl_h[:, T_UDIA:T_UDIA + P] = udia_h
        tbl_h[:, T_NEGI:T_NEGI + P] = negi_h
        tbl_h[:, T_VONE:T_VONE + NKT] = vone
        in_maps.append({
            "xt3": xt3_h,
            "wk8": wk8_h, "wq8": wq8_h, "wv2": wv2_h, "wo2": wo2_h,
            "tbl": tbl_h.astype(BF),
        })
    return in_maps


def kernel(x, Wq, Wk, Wv, Wo, window_size, _trace=False, _trace_kwargs=None):
    assert int(window_size) == W
    if "nc" not in _compiled:
        _compiled["nc"] = _build()
    nc = _compiled["nc"]
    in_maps = _prep_inputs(np.asarray(x), np.asarray(Wq), np.asarray(Wk),
                           np.asarray(Wv), np.asarray(Wo))
    res = run_bass_kernel_spmd(nc, in_maps, core_ids=list(range(NCORES)),
                               trace=_trace, **(_trace_kwargs or {}))
    outp = np.concatenate([res.results[c]["out"] for c in range(NCORES)],
                          axis=0)
    _compiled["last_result"] = res
    return outp.reshape(1, S, DIM).astype(np.float32)


if __name__ == "__main__":
    np.random.seed(0)
    x = np.random.randn(1, S, DIM).astype(np.float32)
    sd = 1.0 / np.sqrt(DIM)
    ws = [np.random.randn(DIM, DIM).astype(np.float32) * sd for _ in range(4)]
    y = kernel(x, *ws, window_size=W)
    print("kernel output", y.shape, y.dtype, np.abs(y).max())
